# revision 2
# baseline (speedup 1.0000x reference)
"""Pointer-network glimpse-attention greedy decode (Kool AttentionModel).

Data-parallel over batch B=256 across 8 NeuronCores (32 rows/core).
K/V/logit keys, masks and the decode scan are fully independent per batch
row; the small DxD projection weights are replicated.

Self-contained: hardcodes shapes B=256, N=1000, D=128, H=8.
kernel(**inputs) -> (log_p [T,B,N] float32, pi [T,B] int32)
"""

import numpy as np

NEG_INF = -1e9
_B, _N, _D, _H = 256, 1000, 128, 8
_M = 8  # NeuronCores

_FN_CACHE = {}


def _build(T):
    import jax
    import jax.numpy as jnp

    dk = _D // _H
    inv_sqrt_dk = 1.0 / np.sqrt(dk)
    inv_sqrt_d = 1.0 / np.sqrt(_D)
    tanh_clipping = 10.0

    def decode(emb, W_node, W_fixed, W_step, W_out):
        Bs, N, D = emb.shape  # 32, 1000, 128 per core
        kvl = emb @ W_node                                  # [Bs,N,3D]
        K, V, logit_K = jnp.split(kvl, 3, axis=-1)
        Kh = K.reshape(Bs, N, _H, dk)
        Vh = V.reshape(Bs, N, _H, dk)
        fixed_ctx = emb.mean(axis=1) @ W_fixed              # [Bs,D]
        batch_idx = jnp.arange(Bs)

        def step(carry, _):
            mask, last_emb = carry
            query = (fixed_ctx + last_emb @ W_step).reshape(Bs, _H, dk)
            compat = jnp.einsum('bhd,bnhd->bhn', query, Kh) * inv_sqrt_dk
            compat = jnp.where(mask[:, None, :], NEG_INF, compat)
            attn = jax.nn.softmax(compat, axis=-1)
            heads = jnp.einsum('bhn,bnhd->bhd', attn, Vh)
            glimpse = heads.reshape(Bs, D) @ W_out
            logits = jnp.einsum('bd,bnd->bn', glimpse, logit_K) * inv_sqrt_d
            logits = tanh_clipping * jnp.tanh(logits)
            logits = jnp.where(mask, NEG_INF, logits)
            log_p = jax.nn.log_softmax(logits, axis=-1)
            # argmax lowers to a 2-operand variadic reduce, which neuronxcc
            # rejects — emulate with max + first-matching-index (same ties).
            m = jnp.max(log_p, axis=-1, keepdims=True)
            iota = jnp.arange(N, dtype=jnp.int32)[None, :]
            sel = jnp.min(jnp.where(log_p == m, iota, N), axis=-1)
            new_mask = mask | (jnp.arange(N)[None, :] == sel[:, None])
            new_last = emb[batch_idx, sel]
            return (new_mask, new_last), (log_p, sel)

        init = (jnp.zeros((Bs, N), dtype=bool), jnp.zeros((Bs, D), emb.dtype))
        _, (log_ps, pis) = jax.lax.scan(step, init, None, length=T)
        return log_ps, pis

    return jax.pmap(decode, in_axes=(0, None, None, None, None))


def kernel(mixed_embeddings, W_node, W_fixed, W_step, W_out, T):
    T = int(T)
    if T not in _FN_CACHE:
        _FN_CACHE[T] = _build(T)
    f = _FN_CACHE[T]

    emb = np.ascontiguousarray(np.asarray(mixed_embeddings, dtype=np.float32))
    emb = emb.reshape(_M, _B // _M, _N, _D)
    wn = np.asarray(W_node, dtype=np.float32)
    wf = np.asarray(W_fixed, dtype=np.float32)
    ws = np.asarray(W_step, dtype=np.float32)
    wo = np.asarray(W_out, dtype=np.float32)

    log_p, pi = f(emb, wn, wf, ws, wo)
    log_p = np.asarray(log_p)                    # [M, T, B/M, N]
    pi = np.asarray(pi)                          # [M, T, B/M]
    log_p = log_p.transpose(1, 0, 2, 3).reshape(T, _B, _N)
    pi = pi.transpose(1, 0, 2).reshape(T, _B).astype(np.int32)
    return log_p, pi
